# revision 21
# baseline (speedup 1.0000x reference)
"""Trainium2 Bass kernel for llama-style attention block (B=4, S=1024, D=4096, H=32).

Strategy: tensor-parallel over heads across 8 NeuronCores (4 heads/core).
 - Host marshals inputs: x transposed to [D, T] (contraction dim on
   partitions), per-core weight slices pre-transposed, q/k weight rows
   deinterleaved (even/odd RoPE pairs -> partition blocks [0:64]/[64:128]),
   everything matmul-facing cast to bf16.
 - Device per core: QKV projections (PE, fp32 accum) -> RoPE (DVE) ->
   attention computed in transposed layout S^T[k,q] so softmax denominators
   come from an all-ones matmul and P@V needs no transposes -> per-batch
   AllGather of context (heads) -> output projection slice -> y columns.
 - Host concatenates the 8 per-core y column slices.
 - The program exploits the mask's tile structure: per 128x512 score tile,
   fully-dead leading columns are skipped in the scores/exp/PV/denominator
   (column-restricted matmuls), and diagonal-crossing tiles are handled by
   a small bf16 0/1-mask multiply on the exp'd SBUF tile instead of an
   fp32 additive mask on PSUM (keeps the DVE off the PSUM-freeing chain).
   Programs are cached per mask structure.

kernel(**inputs) takes the full unsharded inputs as in reference.setup_inputs()
and returns the full [4, 1024, 4096] float32 output.
"""

import math
import sys

import numpy as np
import ml_dtypes

sys.path.insert(0, "/opt/trn_rl_repo")

import concourse.bass as bass  # noqa: E402
import concourse.bass_isa as bass_isa  # noqa: E402
import concourse.mybir as mybir  # noqa: E402
import concourse.tile as tile  # noqa: E402
from concourse import bacc  # noqa: E402
from concourse.bass_utils import run_bass_kernel_spmd  # noqa: E402

P = 128
B, S, D, H = 4, 1024, 4096, 32
T = B * S
HD = 128
NCORES = 8
HPC = H // NCORES          # heads per core = 4
CW = HPC * HD              # per-core width = 512
NDK = D // P               # 32 contraction tiles
TCH = 512                  # token chunk in projection phase
NQ2 = S // 512             # q halves per batch
NKT = S // P               # 8 k tiles per batch

MM = mybir.dt.bfloat16     # matmul operand dtype
F32 = mybir.dt.float32
BF16 = ml_dtypes.bfloat16

AG_GROUPS = [list(range(NCORES))]


def build_program(mask_classes):
    """mask_classes[kt][q2]:
      ('d',)                    fully-masked tile, skipped
      ('z', lo)                 cols [lo:512) live with zero mask
      ('b', lo, mlo, mhi, pid)  cols [lo:512) live; cols [mlo:mhi) need a
                                0/1 multiply with pattern `pid`; rest zero
      ('g', lo)                 general additive mask on cols [lo:512)
    """
    nc = bacc.Bacc("TRN2", target_bir_lowering=False, debug=False,
                   num_devices=NCORES)

    any_g = any(c[0] == 'g' for row in mask_classes for c in row)
    pats = sorted({(c[4], c[3] - c[2]) for row in mask_classes
                   for c in row if c[0] == 'b'})

    # weights and x arrive pre-tiled from the host ([P, NDK, ...]) so
    # every load is partition-contiguous (cheap DMA descriptor generation)
    xT = nc.dram_tensor("xT", [P, NDK, T], MM, kind="ExternalInput").ap()
    wqT = nc.dram_tensor("wqT", [P, NDK, CW], MM, kind="ExternalInput").ap()
    wkT = nc.dram_tensor("wkT", [P, NDK, CW], MM, kind="ExternalInput").ap()
    wvT = nc.dram_tensor("wvT", [P, NDK, CW], MM, kind="ExternalInput").ap()
    woT = nc.dram_tensor("woT", [P, NDK, CW], MM, kind="ExternalInput").ap()
    if any_g:
        maskT = nc.dram_tensor("maskT", [S, S], MM, kind="ExternalInput").ap()
    bmd = {pid: nc.dram_tensor(f"bm{pid}", [P, w], MM,
                               kind="ExternalInput").ap()
           for pid, w in pats}
    cq = nc.dram_tensor("cq", [HD // 2, S], F32, kind="ExternalInput").ap()
    sq = nc.dram_tensor("sq", [HD // 2, S], F32, kind="ExternalInput").ap()
    ck = nc.dram_tensor("ck", [HD // 2, S], F32, kind="ExternalInput").ap()
    sk = nc.dram_tensor("sk", [HD // 2, S], F32, kind="ExternalInput").ap()
    y = nc.dram_tensor("y", [T, CW], F32, kind="ExternalOutput").ap()

    qT_d = nc.dram_tensor("qT_d", [CW, T], MM).ap()
    kT_d = nc.dram_tensor("kT_d", [CW, T], MM).ap()
    v_d = nc.dram_tensor("v_d", [T, CW], MM).ap()
    AG_SPLIT = 4  # collectives; each covers B // AG_SPLIT batches
    BPG = B // AG_SPLIT
    bounce = [nc.dram_tensor(f"bnc{i}", [CW, BPG * S], MM).ap()
              for i in range(AG_SPLIT)]
    ctxT = [nc.dram_tensor(f"ctxT{i}", [D, BPG * S], MM,
                           addr_space="Shared").ap()
            for i in range(AG_SPLIT)]

    sub = mybir.AluOpType.subtract
    add = mybir.AluOpType.add
    mult = mybir.AluOpType.mult
    Exp = mybir.ActivationFunctionType.Exp

    # per q2: kt tiles that contribute (not dead), with their live-col starts
    live_kt = [[(kt, mask_classes[kt][q2][1]) for kt in range(NKT)
                if mask_classes[kt][q2][0] != 'd']
               for q2 in range(NQ2)]
    for q2 in range(NQ2):
        lk = live_kt[q2]
        assert lk, "fully-masked query block unsupported"
        # col-restricted PSUM accumulation: first live kt must cover the
        # union of all later live ranges
        assert lk[0][1] == 0
        assert all(lk[i][1] <= lk[i + 1][1] for i in range(len(lk) - 1))

    # Group live score tiles in pairs sharing one 2-bank PSUM tile and ONE
    # exp op (amortizes the ~0.4us fixed ACT cost). 'g' tiles stay alone.
    live_tiles = [(kt, q2) for kt in range(NKT) for q2 in range(NQ2)
                  if mask_classes[kt][q2][0] != 'd']
    groups = []
    taken = set()
    for kt in range(NKT):  # same-kt pairs (q2=0 + q2=1)
        mem = [(kt, q2) for q2 in range(NQ2)
               if mask_classes[kt][q2][0] in ('z', 'b')]
        if len(mem) == 2:
            groups.append(mem)
            taken.update(mem)
    rest = [t for t in live_tiles if t not in taken
            and mask_classes[t[0]][t[1]][0] in ('z', 'b')]
    for i in range(0, len(rest) - 1, 2):  # consecutive-kt pairs
        groups.append([rest[i], rest[i + 1]])
        taken.update(rest[i:i + 2])
    for t in live_tiles:                  # leftovers / 'g' tiles: alone
        if t not in taken:
            groups.append([t])
    # es slot index per live tile, paired members adjacent
    jmap = {}
    for grp in groups:
        for t in grp:
            jmap[t] = len(jmap)
    NJ = len(jmap)

    with tile.TileContext(nc) as tc:
        with tc.tile_pool(name="persist", bufs=1) as pp:
            # batch-0 attention operands, prefetched during the projection
            # phase so the phase transition has no DMA stall
            qb0 = pp.tile([P, HPC, S], MM)
            kb0 = pp.tile([P, HPC, S], MM)
            # all-ones stationary operand: the denominator matmul yields the
            # per-query softmax sum replicated across all 128 partitions
            ones_sb = pp.tile([P, P], MM)
            nc.any.memset(ones_sb, 1.0)
            bm_sb = {}
            for pid, w in pats:
                t = pp.tile([P, w], MM, tag=f"bm{pid}")
                nc.gpsimd.dma_start(t, bmd[pid])
                bm_sb[pid] = t

            # ---------------- Phase A: projections + RoPE ----------------
            with tc.tile_pool(name="wpool", bufs=1) as wpool, \
                 tc.tile_pool(name="cspool", bufs=1) as cspool, \
                 tc.tile_pool(name="xpool", bufs=2) as xpool, \
                 tc.tile_pool(name="psa", bufs=4, space="PSUM") as psa, \
                 tc.tile_pool(name="stga", bufs=2) as stga, \
                 tc.tile_pool(name="tmpa", bufs=1) as tmpa:

                # first x chunk as 8 independent piece-tiles so the first
                # accumulation group streams behind the DMA instead of
                # waiting for the whole 4MB chunk; weights split across the
                # two HW-DGE queues (sync + scalar) to halve the time until
                # all of chunk 0's operands are resident
                xf = [xpool.tile([P, 16, TCH], MM, tag=f"xf{i}", name=f"xf{i}")
                      for i in range(2)]
                wq_sb = wpool.tile([P, NDK, CW], MM)
                wk_sb = wpool.tile([P, NDK, CW], MM)
                wv_sb = wpool.tile([P, NDK, CW], MM)
                for pc in range(8):
                    dsl = slice(pc * 4, (pc + 1) * 4)
                    nc.sync.dma_start(
                        wq_sb[:, dsl, 0:HD], wqT[:, dsl, 0:HD])
                    nc.scalar.dma_start(
                        xf[pc // 4][:, 4 * (pc % 4):4 * (pc % 4 + 1), :],
                        xT[:, dsl, 0:TCH])
                nc.sync.dma_start(wk_sb[:, :, 0:HD], wkT[:, :, 0:HD])

                hs1 = slice(HD, 2 * HD)
                nc.sync.dma_start(wq_sb[:, :, hs1], wqT[:, :, hs1])
                nc.sync.dma_start(wk_sb[:, :, hs1], wkT[:, :, hs1])

                cq_sb = cspool.tile([HD // 2, S], F32)
                sq_sb = cspool.tile([HD // 2, S], F32)
                ck_sb = cspool.tile([HD // 2, S], F32)
                sk_sb = cspool.tile([HD // 2, S], F32)
                nc.sync.dma_start(cq_sb, cq)
                nc.sync.dma_start(sq_sb, sq)
                nc.sync.dma_start(ck_sb, ck)
                nc.sync.dma_start(sk_sb, sk)

                hs2 = slice(2 * HD, 3 * HD)
                nc.sync.dma_start(wq_sb[:, :, hs2], wqT[:, :, hs2])
                nc.sync.dma_start(wk_sb[:, :, hs2], wkT[:, :, hs2])
                hs3 = slice(3 * HD, 4 * HD)
                nc.scalar.dma_start(wq_sb[:, :, hs3], wqT[:, :, hs3])
                nc.scalar.dma_start(wk_sb[:, :, hs3], wkT[:, :, hs3])

                nc.sync.dma_start(
                    wv_sb[:, :, 0:CW // 2], wvT[:, :, 0:CW // 2])
                nc.scalar.dma_start(
                    wv_sb[:, :, CW // 2:CW], wvT[:, :, CW // 2:CW])

                xs_next = xf
                for tch in range(T // TCH):
                    t0 = tch * TCH
                    s0 = t0 % S
                    xs = xs_next

                    # q/k for the 4 local heads; RoPE on psum eviction
                    # (last chunk: v first so its PSUM banks drain early)
                    for part in ((1, 0) if tch == T // TCH - 1 else (0, 1)):
                     if part == 0:
                      for h in range(HPC):
                        for wsb, c_sb, s_sb, dst in (
                                (wq_sb, cq_sb, sq_sb, qT_d),
                                (wk_sb, ck_sb, sk_sb, kT_d)):
                            ps = psa.tile([P, TCH], F32, tag="qk", bufs=6)
                            for dk in range(NDK):
                                nc.tensor.matmul(
                                    ps, lhsT=wsb[:, dk, h * HD:(h + 1) * HD],
                                    rhs=xs[dk // 16][:, dk % 16, :],
                                    start=(dk == 0), stop=(dk == NDK - 1))
                            a = ps[0:HD // 2]
                            bb = ps[HD // 2:P]
                            cc = c_sb[:, s0:s0 + TCH]
                            ss = s_sb[:, s0:s0 + TCH]
                            out = stga.tile([P, TCH], MM, tag="qkstage")
                            t1 = tmpa.tile([HD // 2, TCH], F32, tag="t1")
                            t2 = tmpa.tile([HD // 2, TCH], F32, tag="t2")
                            nc.vector.tensor_tensor(t1, a, cc, mult)
                            nc.vector.tensor_tensor(t2, bb, ss, mult)
                            nc.vector.tensor_tensor(out[0:HD // 2], t1, t2, sub)
                            t3 = tmpa.tile([HD // 2, TCH], F32, tag="t1")
                            t4 = tmpa.tile([HD // 2, TCH], F32, tag="t2")
                            nc.vector.tensor_tensor(t3, a, ss, mult)
                            nc.vector.tensor_tensor(t4, bb, cc, mult)
                            nc.vector.tensor_tensor(out[HD // 2:P], t3, t4, add)
                            nc.scalar.dma_start(
                                dst[h * HD:(h + 1) * HD, t0:t0 + TCH], out)

                     else:
                      # v for the 4 local heads (natural [t, hd] layout);
                      # evict on the otherwise-idle scalar engine
                      for tt in range(TCH // P):
                        ps = psa.tile([P, CW], F32, tag="v", bufs=2)
                        for dk in range(NDK):
                            nc.tensor.matmul(
                                ps,
                                lhsT=xs[dk // 16][:, dk % 16,
                                                  tt * P:(tt + 1) * P],
                                rhs=wv_sb[:, dk, :],
                                start=(dk == 0), stop=(dk == NDK - 1))
                        vo = stga.tile([P, CW], MM, tag="vstage")
                        nc.scalar.copy(vo, ps)
                        nc.scalar.dma_start(
                            v_d[t0 + tt * P:t0 + (tt + 1) * P, :], vo)

                    if tch == 1:
                        # batch-0 q/k just landed in DRAM: prefetch the
                        # attention-layout tiles while projections continue
                        nc.scalar.dma_start(
                            qb0, qT_d[:, 0:S]
                            .rearrange("(h p) t -> p h t", p=P))
                        nc.scalar.dma_start(
                            kb0, kT_d[:, 0:S]
                            .rearrange("(h p) t -> p h t", p=P))
                    if tch + 1 < T // TCH:
                        # next chunk's x, behind this chunk's writes in the
                        # queue so output slots recycle promptly
                        t0n = (tch + 1) * TCH
                        xs_next = [xpool.tile([P, 16, TCH], MM,
                                              tag=f"xf{i}", name=f"xc{i}")
                                   for i in range(2)]
                        for i in range(2):
                            nc.scalar.dma_start(
                                xs_next[i],
                                xT[:, 16 * i:16 * (i + 1), t0n:t0n + TCH])

            # ---------------- Phase B/C: attention + AllGather + wo ------
            with tc.tile_pool(name="mpool", bufs=1) as mpool, \
                 tc.tile_pool(name="qkvp", bufs=2) as qkvp, \
                 tc.tile_pool(name="esp", bufs=3) as esp, \
                 tc.tile_pool(name="psb", bufs=2, space="PSUM") as psb, \
                 tc.tile_pool(name="tmpb", bufs=4) as tmpb, \
                 tc.tile_pool(name="stgb", bufs=4) as stgb, \
                 tc.tile_pool(name="cxp", bufs=2) as cxp:

                if any_g:
                    mask_sb = mpool.tile([P, NKT, S], MM)
                    nc.sync.dma_start(
                        mask_sb, maskT.rearrange("(kt p) q -> p kt q", p=P))
                wo_sb = mpool.tile([P, NDK, CW], MM)

                def qkv_load(b):
                    # whole-batch loads on the scalar HW queue (the sync
                    # queue carries the projection writes + bounce traffic)
                    qb = qkvp.tile([P, HPC, S], MM, tag="qb")
                    kb = qkvp.tile([P, HPC, S], MM, tag="kb")
                    vb = qkvp.tile([P, NKT, CW], MM, tag="vb")
                    nc.sync.dma_start(
                        qb, qT_d[:, b * S:(b + 1) * S]
                        .rearrange("(h p) t -> p h t", p=P))
                    nc.sync.dma_start(
                        kb, kT_d[:, b * S:(b + 1) * S]
                        .rearrange("(h p) t -> p h t", p=P))
                    nc.sync.dma_start(
                        vb, v_d[b * S:(b + 1) * S, :]
                        .rearrange("(kt p) w -> p kt w", p=P))
                    return qb, kb, vb

                def attn_batch(b, qb, kb, vb):
                    # pass 1: scores + exp for all heads (PE runs ahead);
                    # es is a flat [P, NJ*512] tile, one 512-col slot per
                    # live score tile, group members in adjacent slots so
                    # one exp covers a whole group
                    es_h = []
                    for h in range(HPC):
                        es = esp.tile([P, NJ * 512], MM, tag="es")
                        es_h.append(es)
                        for grp in groups:
                            ps_s = psb.tile([P, 1024], F32, tag="sc",
                                            bufs=2)
                            for idx, (kt, q2) in enumerate(grp):
                                cls = mask_classes[kt][q2]
                                lo = cls[1]
                                nc.tensor.matmul(
                                    ps_s[:, idx * 512 + lo:(idx + 1) * 512],
                                    lhsT=kb[:, h, kt * P:(kt + 1) * P],
                                    rhs=qb[:, h, q2 * 512 + lo:
                                           (q2 + 1) * 512],
                                    start=True, stop=True)
                            cls0 = mask_classes[grp[0][0]][grp[0][1]]
                            j0 = jmap[grp[0]]
                            if cls0[0] == 'g':
                                (kt, q2), = grp
                                lo = cls0[1]
                                tmp = tmpb.tile([P, 512], F32, tag="sadd")
                                nc.vector.tensor_tensor(
                                    tmp[:, lo:512], ps_s[:, lo:512],
                                    mask_sb[:, kt,
                                            q2 * 512 + lo:(q2 + 1) * 512],
                                    add)
                                nc.scalar.activation(
                                    es[:, j0 * 512 + lo:(j0 + 1) * 512],
                                    tmp[:, lo:512], Exp)
                            else:
                                lo0 = cls0[1]
                                w = len(grp) * 512
                                nc.scalar.activation(
                                    es[:, j0 * 512 + lo0:j0 * 512 + w],
                                    ps_s[:, lo0:w], Exp)
                                for (kt, q2) in grp:
                                    cls = mask_classes[kt][q2]
                                    if cls[0] == 'b':
                                        _, _, mlo, mhi, pid = cls
                                        j = jmap[(kt, q2)]
                                        msl = slice(j * 512 + mlo,
                                                    j * 512 + mhi)
                                        nc.vector.tensor_tensor(
                                            es[:, msl], es[:, msl],
                                            bm_sb[pid], mult)
                    # pass 2: P@V + denominators (ones matmul) + normalize
                    for h in range(HPC):
                        hs = slice(h * HD, (h + 1) * HD)
                        es = es_h[h]
                        for q2 in range(NQ2):
                            lk = live_kt[q2]
                            ps_o = psb.tile([P, 512], F32, tag="ot", bufs=2)
                            for i, (kt, lo) in enumerate(lk):
                                j = jmap[(kt, q2)]
                                nc.tensor.matmul(
                                    ps_o[:, lo:512], lhsT=vb[:, kt, hs],
                                    rhs=es[:, j * 512 + lo:(j + 1) * 512],
                                    start=(i == 0), stop=(i == len(lk) - 1))
                            ps_m = psb.tile([P, 512], F32, tag="sum", bufs=2)
                            for i, (kt, lo) in enumerate(lk):
                                j = jmap[(kt, q2)]
                                nc.tensor.matmul(
                                    ps_m[:, lo:512], lhsT=ones_sb,
                                    rhs=es[:, j * 512 + lo:(j + 1) * 512],
                                    start=(i == 0), stop=(i == len(lk) - 1))
                            rec = tmpb.tile([P, 512], F32, tag="rec", bufs=4)
                            nc.vector.reciprocal_approx_fast(rec, ps_m)
                            ob = stgb.tile([P, 512], MM, tag="ob", bufs=8)
                            nc.vector.tensor_tensor(ob, ps_o, rec, mult)
                            nc.sync.dma_start(
                                bounce[b // BPG][h * HD:(h + 1) * HD,
                                                 (b % BPG) * S + q2 * 512:
                                                 (b % BPG) * S + (q2 + 1) * 512],
                                ob)

                def wo_batch(b):
                    # paired token tiles: 512B DMA lines on ctx gather reads
                    for tt in range(0, S // P, 2):
                        c0 = (b % BPG) * S + tt * P
                        cx = cxp.tile([P, NDK, 2 * P], MM, tag="cx")
                        nc.scalar.dma_start(
                            cx, ctxT[b // BPG][:, c0:c0 + 2 * P]
                            .rearrange("(o p) t -> p o t", p=P))
                        ps_y0 = psb.tile([P, CW], F32, tag="ot", bufs=2)
                        ps_y1 = psb.tile([P, CW], F32, tag="sum", bufs=2)
                        for dk in range(NDK):
                            nc.tensor.matmul(
                                ps_y0, lhsT=cx[:, dk, 0:P],
                                rhs=wo_sb[:, dk, :],
                                start=(dk == 0), stop=(dk == NDK - 1))
                            nc.tensor.matmul(
                                ps_y1, lhsT=cx[:, dk, P:2 * P],
                                rhs=wo_sb[:, dk, :],
                                start=(dk == 0), stop=(dk == NDK - 1))
                        for j, ps_y in enumerate((ps_y0, ps_y1)):
                            yo = stgb.tile([P, CW], F32, tag="yo", bufs=2)
                            nc.scalar.copy(yo, ps_y)
                            nc.sync.dma_start(
                                y[b * S + (tt + j) * P:
                                  b * S + (tt + j + 1) * P, :], yo)

                def allgather(i):
                    nc.gpsimd.collective_compute(
                        "AllGather", mybir.AluOpType.bypass,
                        replica_groups=AG_GROUPS,
                        ins=[bounce[i]], outs=[ctxT[i]])

                # software-pipeline: per-batch AllGathers (serialized on the
                # collective stream, ~55us each) hidden under the remaining
                # attention batches and the wo projections. All qkv loads
                # are prefetched a batch ahead on the scalar queue.
                vb0 = qkvp.tile([P, NKT, CW], MM, tag="vb")
                nc.sync.dma_start(
                    vb0, v_d[0:S, :].rearrange("(kt p) w -> p kt w", p=P))
                t1 = qkv_load(1)
                nc.sync.dma_start(wo_sb, woT)
                attn_batch(0, qb0, kb0, vb0)
                allgather(0)
                t2 = qkv_load(2)
                attn_batch(1, *t1)
                allgather(1)
                t3 = qkv_load(3)
                attn_batch(2, *t2)
                allgather(2)
                attn_batch(3, *t3)
                allgather(3)
                wo_batch(0)
                wo_batch(1)
                wo_batch(2)
                wo_batch(3)

    nc.compile()
    return nc


_NC_CACHE = {}


def _get_nc(mask_classes):
    key = tuple(map(tuple, mask_classes))
    if key not in _NC_CACHE:
        _NC_CACHE[key] = build_program(mask_classes)
    return _NC_CACHE[key]


def _classify_mask(maskT_f32):
    """Per score tile [kt*128:(kt+1)*128, q2*512:(q2+1)*512] of mask^T,
    return the class tuple (see build_program) plus 0/1 patterns."""
    classes = []
    patterns = {}
    pat_ids = {}
    for kt in range(NKT):
        row = []
        for q2 in range(NQ2):
            t = maskT_f32[kt * P:(kt + 1) * P, q2 * 512:(q2 + 1) * 512]
            dead_col = np.all(t <= -1e30, axis=0)   # [512]
            if dead_col.all():
                row.append(('d',))
                continue
            live = ~dead_col
            lo = int(np.argmax(live))
            if not live[lo:].all():
                # non-prefix deadness: general fallback
                row.append(('g', 0))
                continue
            sub = t[:, lo:]
            if np.all(sub == 0.0):
                row.append(('z', lo))
                continue
            is_neg = sub <= -1e30
            if not np.all(is_neg | (sub == 0.0)):
                row.append(('g', lo))
                continue
            mixed = is_neg.any(axis=0)              # cols needing 0/1 mask
            m_idx = np.nonzero(mixed)[0]
            mlo, mhi = int(m_idx[0]), int(m_idx[-1]) + 1
            if mixed[mlo:mhi].sum() != mhi - mlo:
                row.append(('g', lo))               # non-contiguous mixed
                continue
            pat = (~is_neg[:, mlo:mhi]).astype(np.float32)
            key = pat.tobytes() + bytes([mhi - mlo])
            if key not in pat_ids:
                pat_ids[key] = len(pat_ids)
                patterns[pat_ids[key]] = np.ascontiguousarray(
                    pat.astype(BF16))
            row.append(('b', lo, lo + mlo, lo + mhi, pat_ids[key]))
        classes.append(row)
    return classes, patterns


def _prep_inputs(x, freqs_cos, freqs_sin, mask, wq, wk, wv, wo):
    """Host-side sharding/layout marshaling. Returns per-core input maps."""
    x = np.asarray(x, np.float32).reshape(T, D)
    xT = np.ascontiguousarray(
        x.T.reshape(NDK, P, T).transpose(1, 0, 2).astype(BF16))

    cos = np.asarray(freqs_cos, np.float32)
    sin = np.asarray(freqs_sin, np.float32)
    qscale = 1.0 / math.sqrt(HD)
    cqh = np.ascontiguousarray(cos.T * qscale).astype(np.float32)
    sqh = np.ascontiguousarray(sin.T * qscale).astype(np.float32)
    ckh = np.ascontiguousarray(cos.T).astype(np.float32)
    skh = np.ascontiguousarray(sin.T).astype(np.float32)

    m = np.asarray(mask, np.float32).reshape(S, S)
    mT = np.ascontiguousarray(m.T)
    classes, patterns = _classify_mask(mT)
    any_g = any(c[0] == 'g' for row in classes for c in row)
    maskTb = np.ascontiguousarray(np.maximum(mT, -60000.0).astype(BF16))

    # deinterleave RoPE pairs within each head's weight rows: row order
    # [0,2,...,126,1,3,...,127] so pairs land in partition blocks.
    perm = np.concatenate([np.arange(0, HD, 2), np.arange(1, HD, 2)])

    wq = np.asarray(wq, np.float32)
    wk = np.asarray(wk, np.float32)
    wv = np.asarray(wv, np.float32)
    wo = np.asarray(wo, np.float32)

    in_maps = []
    for c in range(NCORES):
        r0, r1 = c * CW, (c + 1) * CW
        wq_c = wq[r0:r1].reshape(HPC, HD, D)[:, perm, :].reshape(CW, D)
        wk_c = wk[r0:r1].reshape(HPC, HD, D)[:, perm, :].reshape(CW, D)
        wv_c = wv[r0:r1]
        wo_c = wo[r0:r1]
        im = {
            "xT": xT,
            "wqT": _pretile(wq_c.T),
            "wkT": _pretile(wk_c.T),
            "wvT": _pretile(wv_c.T),
            "woT": _pretile(wo_c.T),
            "cq": cqh, "sq": sqh, "ck": ckh, "sk": skh,
        }
        if any_g:
            im["maskT"] = maskTb
        for pid, pat in patterns.items():
            im[f"bm{pid}"] = pat
        in_maps.append(im)
    return in_maps, classes


def _pretile(wT):
    """[D, CW] -> [P, NDK, CW] with [p, o, m] = wT[o*P + p, m]."""
    return np.ascontiguousarray(
        wT.reshape(NDK, P, CW).transpose(1, 0, 2).astype(BF16))


def kernel(x, start_pos, freqs_cos, freqs_sin, mask, wq, wk, wv, wo,
           cache_k, cache_v, _trace=False):
    assert int(start_pos) == 0, "kernel specialized for start_pos=0"
    in_maps, classes = _prep_inputs(x, freqs_cos, freqs_sin, mask,
                                    wq, wk, wv, wo)
    nc = _get_nc(classes)
    res = run_bass_kernel_spmd(nc, in_maps, list(range(NCORES)), trace=_trace)
    kernel.last_results = res
    yfull = np.concatenate([res.results[c]["y"] for c in range(NCORES)],
                           axis=1)
    return yfull.reshape(B, S, D).astype(np.float32)


# revision 22
# speedup vs baseline: 1.0233x; 1.0233x over previous
"""Trainium2 Bass kernel for llama-style attention block (B=4, S=1024, D=4096, H=32).

Strategy: tensor-parallel over heads across 8 NeuronCores (4 heads/core).
 - Host marshals inputs: x transposed to [D, T] (contraction dim on
   partitions), per-core weight slices pre-transposed, q/k weight rows
   deinterleaved (even/odd RoPE pairs -> partition blocks [0:64]/[64:128]),
   everything matmul-facing cast to bf16.
 - Device per core: QKV projections (PE, fp32 accum) -> RoPE (DVE) ->
   attention computed in transposed layout S^T[k,q] so softmax denominators
   come from an all-ones matmul and P@V needs no transposes -> per-batch
   AllGather of context (heads) -> output projection slice -> y columns.
 - Host concatenates the 8 per-core y column slices.
 - The program exploits the mask's tile structure: per 128x512 score tile,
   fully-dead leading columns are skipped in the scores/exp/PV/denominator
   (column-restricted matmuls), and diagonal-crossing tiles are handled by
   a small bf16 0/1-mask multiply on the exp'd SBUF tile instead of an
   fp32 additive mask on PSUM (keeps the DVE off the PSUM-freeing chain).
   Programs are cached per mask structure.

kernel(**inputs) takes the full unsharded inputs as in reference.setup_inputs()
and returns the full [4, 1024, 4096] float32 output.
"""

import math
import sys

import numpy as np
import ml_dtypes

sys.path.insert(0, "/opt/trn_rl_repo")

import concourse.bass as bass  # noqa: E402
import concourse.bass_isa as bass_isa  # noqa: E402
import concourse.mybir as mybir  # noqa: E402
import concourse.tile as tile  # noqa: E402
from concourse import bacc  # noqa: E402
from concourse.bass_utils import run_bass_kernel_spmd  # noqa: E402

P = 128
B, S, D, H = 4, 1024, 4096, 32
T = B * S
HD = 128
NCORES = 8
HPC = H // NCORES          # heads per core = 4
CW = HPC * HD              # per-core width = 512
NDK = D // P               # 32 contraction tiles
TCH = 512                  # token chunk in projection phase
NQ2 = S // 512             # q halves per batch
NKT = S // P               # 8 k tiles per batch

MM = mybir.dt.bfloat16     # matmul operand dtype
F32 = mybir.dt.float32
BF16 = ml_dtypes.bfloat16

AG_GROUPS = [list(range(NCORES))]


def build_program(mask_classes):
    """mask_classes[kt][q2]:
      ('d',)                    fully-masked tile, skipped
      ('z', lo)                 cols [lo:512) live with zero mask
      ('b', lo, mlo, mhi, pid)  cols [lo:512) live; cols [mlo:mhi) need a
                                0/1 multiply with pattern `pid`; rest zero
      ('g', lo)                 general additive mask on cols [lo:512)
    """
    nc = bacc.Bacc("TRN2", target_bir_lowering=False, debug=False,
                   num_devices=NCORES)

    any_g = any(c[0] == 'g' for row in mask_classes for c in row)
    pats = sorted({(c[4], c[3] - c[2]) for row in mask_classes
                   for c in row if c[0] == 'b'})

    # weights and x arrive pre-tiled from the host ([P, NDK, ...]) so
    # every load is partition-contiguous (cheap DMA descriptor generation)
    xT = nc.dram_tensor("xT", [P, NDK, T], MM, kind="ExternalInput").ap()
    wqT = nc.dram_tensor("wqT", [P, NDK, CW], MM, kind="ExternalInput").ap()
    wkT = nc.dram_tensor("wkT", [P, NDK, CW], MM, kind="ExternalInput").ap()
    wvT = nc.dram_tensor("wvT", [P, NDK, CW], MM, kind="ExternalInput").ap()
    woT = nc.dram_tensor("woT", [P, NDK, CW], MM, kind="ExternalInput").ap()
    if any_g:
        maskT = nc.dram_tensor("maskT", [S, S], MM, kind="ExternalInput").ap()
    bmd = {pid: nc.dram_tensor(f"bm{pid}", [P, w], MM,
                               kind="ExternalInput").ap()
           for pid, w in pats}
    cq = nc.dram_tensor("cq", [HD // 2, S], F32, kind="ExternalInput").ap()
    sq = nc.dram_tensor("sq", [HD // 2, S], F32, kind="ExternalInput").ap()
    ck = nc.dram_tensor("ck", [HD // 2, S], F32, kind="ExternalInput").ap()
    sk = nc.dram_tensor("sk", [HD // 2, S], F32, kind="ExternalInput").ap()
    y = nc.dram_tensor("y", [T, CW], F32, kind="ExternalOutput").ap()

    qT_d = nc.dram_tensor("qT_d", [CW, T], MM).ap()
    kT_d = nc.dram_tensor("kT_d", [CW, T], MM).ap()
    v_d = nc.dram_tensor("v_d", [T, CW], MM).ap()
    AG_SPLIT = 4  # collectives; each covers B // AG_SPLIT batches
    BPG = B // AG_SPLIT
    bounce = [nc.dram_tensor(f"bnc{i}", [CW, BPG * S], MM).ap()
              for i in range(AG_SPLIT)]
    ctxT = [nc.dram_tensor(f"ctxT{i}", [D, BPG * S], MM,
                           addr_space="Shared").ap()
            for i in range(AG_SPLIT)]

    sub = mybir.AluOpType.subtract
    add = mybir.AluOpType.add
    mult = mybir.AluOpType.mult
    Exp = mybir.ActivationFunctionType.Exp

    # per q2: kt tiles that contribute (not dead), with their live-col starts
    live_kt = [[(kt, mask_classes[kt][q2][1]) for kt in range(NKT)
                if mask_classes[kt][q2][0] != 'd']
               for q2 in range(NQ2)]
    for q2 in range(NQ2):
        lk = live_kt[q2]
        assert lk, "fully-masked query block unsupported"
        # col-restricted PSUM accumulation: first live kt must cover the
        # union of all later live ranges
        assert lk[0][1] == 0
        assert all(lk[i][1] <= lk[i + 1][1] for i in range(len(lk) - 1))

    # Group live score tiles in pairs sharing one 2-bank PSUM tile and ONE
    # exp op (amortizes the ~0.4us fixed ACT cost). 'g' tiles stay alone.
    live_tiles = [(kt, q2) for kt in range(NKT) for q2 in range(NQ2)
                  if mask_classes[kt][q2][0] != 'd']
    groups = []
    taken = set()
    for kt in range(NKT):  # same-kt pairs (q2=0 + q2=1)
        mem = [(kt, q2) for q2 in range(NQ2)
               if mask_classes[kt][q2][0] in ('z', 'b')]
        if len(mem) == 2:
            groups.append(mem)
            taken.update(mem)
    rest = [t for t in live_tiles if t not in taken
            and mask_classes[t[0]][t[1]][0] in ('z', 'b')]
    for i in range(0, len(rest) - 1, 2):  # consecutive-kt pairs
        groups.append([rest[i], rest[i + 1]])
        taken.update(rest[i:i + 2])
    for t in live_tiles:                  # leftovers / 'g' tiles: alone
        if t not in taken:
            groups.append([t])
    # es slot index per live tile, paired members adjacent
    jmap = {}
    for grp in groups:
        for t in grp:
            jmap[t] = len(jmap)
    NJ = len(jmap)

    with tile.TileContext(nc) as tc:
        with tc.tile_pool(name="persist", bufs=1) as pp:
            # batch-0 attention operands, prefetched during the projection
            # phase so the phase transition has no DMA stall
            qb0 = pp.tile([P, HPC, S], MM)
            kb0 = pp.tile([P, HPC, S], MM)
            # all-ones stationary operand: the denominator matmul yields the
            # per-query softmax sum replicated across all 128 partitions
            ones_sb = pp.tile([P, P], MM)
            nc.any.memset(ones_sb, 1.0)
            bm_sb = {}
            for pid, w in pats:
                t = pp.tile([P, w], MM, tag=f"bm{pid}")
                nc.gpsimd.dma_start(t, bmd[pid])
                bm_sb[pid] = t

            # ---------------- Phase A: projections + RoPE ----------------
            with tc.tile_pool(name="wpool", bufs=1) as wpool, \
                 tc.tile_pool(name="cspool", bufs=1) as cspool, \
                 tc.tile_pool(name="xpool", bufs=2) as xpool, \
                 tc.tile_pool(name="psa", bufs=4, space="PSUM") as psa, \
                 tc.tile_pool(name="stga", bufs=2) as stga, \
                 tc.tile_pool(name="tmpa", bufs=1) as tmpa:

                # first x chunk as 8 independent piece-tiles so the first
                # accumulation group streams behind the DMA instead of
                # waiting for the whole 4MB chunk; weights split across the
                # two HW-DGE queues (sync + scalar) to halve the time until
                # all of chunk 0's operands are resident
                xf = [xpool.tile([P, 16, TCH], MM, tag=f"xf{i}", name=f"xf{i}")
                      for i in range(2)]
                wq_sb = wpool.tile([P, NDK, CW], MM)
                wk_sb = wpool.tile([P, NDK, CW], MM)
                wv_sb = wpool.tile([P, NDK, CW], MM)
                for pc in range(8):
                    dsl = slice(pc * 4, (pc + 1) * 4)
                    nc.sync.dma_start(
                        wq_sb[:, dsl, 0:HD], wqT[:, dsl, 0:HD])
                    nc.scalar.dma_start(
                        xf[pc // 4][:, 4 * (pc % 4):4 * (pc % 4 + 1), :],
                        xT[:, dsl, 0:TCH])
                nc.sync.dma_start(wk_sb[:, :, 0:HD], wkT[:, :, 0:HD])

                hs1 = slice(HD, 2 * HD)
                nc.sync.dma_start(wq_sb[:, :, hs1], wqT[:, :, hs1])
                nc.sync.dma_start(wk_sb[:, :, hs1], wkT[:, :, hs1])

                cq_sb = cspool.tile([HD // 2, S], F32)
                sq_sb = cspool.tile([HD // 2, S], F32)
                ck_sb = cspool.tile([HD // 2, S], F32)
                sk_sb = cspool.tile([HD // 2, S], F32)
                nc.sync.dma_start(cq_sb, cq)
                nc.sync.dma_start(sq_sb, sq)
                nc.sync.dma_start(ck_sb, ck)
                nc.sync.dma_start(sk_sb, sk)

                hs2 = slice(2 * HD, 3 * HD)
                nc.sync.dma_start(wq_sb[:, :, hs2], wqT[:, :, hs2])
                nc.sync.dma_start(wk_sb[:, :, hs2], wkT[:, :, hs2])
                hs3 = slice(3 * HD, 4 * HD)
                nc.scalar.dma_start(wq_sb[:, :, hs3], wqT[:, :, hs3])
                nc.scalar.dma_start(wk_sb[:, :, hs3], wkT[:, :, hs3])

                nc.sync.dma_start(
                    wv_sb[:, :, 0:CW // 2], wvT[:, :, 0:CW // 2])
                nc.scalar.dma_start(
                    wv_sb[:, :, CW // 2:CW], wvT[:, :, CW // 2:CW])

                xs_next = xf
                for tch in range(T // TCH):
                    t0 = tch * TCH
                    s0 = t0 % S
                    xs = xs_next

                    # q/k for the 4 local heads; RoPE on psum eviction
                    # (last chunk: v first so its PSUM banks drain early)
                    for part in ((1, 0) if tch == T // TCH - 1 else (0, 1)):
                     if part == 0:
                      for h in range(HPC):
                        for wsb, c_sb, s_sb, dst in (
                                (wq_sb, cq_sb, sq_sb, qT_d),
                                (wk_sb, ck_sb, sk_sb, kT_d)):
                            ps = psa.tile([P, TCH], F32, tag="qk", bufs=6)
                            for dk in range(NDK):
                                nc.tensor.matmul(
                                    ps, lhsT=wsb[:, dk, h * HD:(h + 1) * HD],
                                    rhs=xs[dk // 16][:, dk % 16, :],
                                    start=(dk == 0), stop=(dk == NDK - 1))
                            a = ps[0:HD // 2]
                            bb = ps[HD // 2:P]
                            cc = c_sb[:, s0:s0 + TCH]
                            ss = s_sb[:, s0:s0 + TCH]
                            out = stga.tile([P, TCH], MM, tag="qkstage")
                            t1 = tmpa.tile([HD // 2, TCH], F32, tag="t1")
                            t2 = tmpa.tile([HD // 2, TCH], F32, tag="t2")
                            nc.vector.tensor_tensor(t1, a, cc, mult)
                            nc.vector.tensor_tensor(t2, bb, ss, mult)
                            nc.vector.tensor_tensor(out[0:HD // 2], t1, t2, sub)
                            t3 = tmpa.tile([HD // 2, TCH], F32, tag="t1")
                            t4 = tmpa.tile([HD // 2, TCH], F32, tag="t2")
                            nc.vector.tensor_tensor(t3, a, ss, mult)
                            nc.vector.tensor_tensor(t4, bb, cc, mult)
                            nc.vector.tensor_tensor(out[HD // 2:P], t3, t4, add)
                            nc.scalar.dma_start(
                                dst[h * HD:(h + 1) * HD, t0:t0 + TCH], out)

                     else:
                      # v for the 4 local heads (natural [t, hd] layout);
                      # evict on the otherwise-idle scalar engine
                      for tt in range(TCH // P):
                        ps = psa.tile([P, CW], F32, tag="v", bufs=2)
                        for dk in range(NDK):
                            nc.tensor.matmul(
                                ps,
                                lhsT=xs[dk // 16][:, dk % 16,
                                                  tt * P:(tt + 1) * P],
                                rhs=wv_sb[:, dk, :],
                                start=(dk == 0), stop=(dk == NDK - 1))
                        vo = stga.tile([P, CW], MM, tag="vstage")
                        nc.scalar.copy(vo, ps)
                        nc.scalar.dma_start(
                            v_d[t0 + tt * P:t0 + (tt + 1) * P, :], vo)

                    if tch == 1:
                        # batch-0 q/k just landed in DRAM: prefetch the
                        # attention-layout tiles while projections continue
                        nc.scalar.dma_start(
                            qb0, qT_d[:, 0:S]
                            .rearrange("(h p) t -> p h t", p=P))
                        nc.scalar.dma_start(
                            kb0, kT_d[:, 0:S]
                            .rearrange("(h p) t -> p h t", p=P))
                    if tch + 1 < T // TCH:
                        # next chunk's x, behind this chunk's writes in the
                        # queue so output slots recycle promptly
                        t0n = (tch + 1) * TCH
                        xs_next = [xpool.tile([P, 16, TCH], MM,
                                              tag=f"xf{i}", name=f"xc{i}")
                                   for i in range(2)]
                        for i in range(2):
                            nc.scalar.dma_start(
                                xs_next[i],
                                xT[:, 16 * i:16 * (i + 1), t0n:t0n + TCH])

            # ---------------- Phase B/C: attention + AllGather + wo ------
            with tc.tile_pool(name="mpool", bufs=1) as mpool, \
                 tc.tile_pool(name="qkvp", bufs=2) as qkvp, \
                 tc.tile_pool(name="esp", bufs=3) as esp, \
                 tc.tile_pool(name="psb", bufs=2, space="PSUM") as psb, \
                 tc.tile_pool(name="tmpb", bufs=4) as tmpb, \
                 tc.tile_pool(name="stgb", bufs=4) as stgb, \
                 tc.tile_pool(name="cxp", bufs=2) as cxp:

                if any_g:
                    mask_sb = mpool.tile([P, NKT, S], MM)
                    nc.sync.dma_start(
                        mask_sb, maskT.rearrange("(kt p) q -> p kt q", p=P))
                wo_sb = mpool.tile([P, NDK, CW], MM)

                def qkv_load(b):
                    # whole-batch loads on the scalar HW queue (the sync
                    # queue carries the projection writes + bounce traffic)
                    qb = qkvp.tile([P, HPC, S], MM, tag="qb")
                    kb = qkvp.tile([P, HPC, S], MM, tag="kb")
                    vb = qkvp.tile([P, NKT, CW], MM, tag="vb")
                    nc.sync.dma_start(
                        qb, qT_d[:, b * S:(b + 1) * S]
                        .rearrange("(h p) t -> p h t", p=P))
                    nc.sync.dma_start(
                        kb, kT_d[:, b * S:(b + 1) * S]
                        .rearrange("(h p) t -> p h t", p=P))
                    nc.sync.dma_start(
                        vb, v_d[b * S:(b + 1) * S, :]
                        .rearrange("(kt p) w -> p kt w", p=P))
                    return qb, kb, vb

                def attn_batch(b, qb, kb, vb):
                    # pass 1: scores + exp for all heads (PE runs ahead);
                    # es is a flat [P, NJ*512] tile, one 512-col slot per
                    # live score tile, group members in adjacent slots so
                    # one exp covers a whole group
                    es_h = []
                    for h in range(HPC):
                        es = esp.tile([P, NJ * 512], MM, tag="es")
                        es_h.append(es)
                        for grp in groups:
                            ps_s = psb.tile([P, 1024], F32, tag="sc",
                                            bufs=2)
                            for idx, (kt, q2) in enumerate(grp):
                                cls = mask_classes[kt][q2]
                                lo = cls[1]
                                nc.tensor.matmul(
                                    ps_s[:, idx * 512 + lo:(idx + 1) * 512],
                                    lhsT=kb[:, h, kt * P:(kt + 1) * P],
                                    rhs=qb[:, h, q2 * 512 + lo:
                                           (q2 + 1) * 512],
                                    start=True, stop=True)
                            cls0 = mask_classes[grp[0][0]][grp[0][1]]
                            j0 = jmap[grp[0]]
                            if cls0[0] == 'g':
                                (kt, q2), = grp
                                lo = cls0[1]
                                tmp = tmpb.tile([P, 512], F32, tag="sadd")
                                nc.vector.tensor_tensor(
                                    tmp[:, lo:512], ps_s[:, lo:512],
                                    mask_sb[:, kt,
                                            q2 * 512 + lo:(q2 + 1) * 512],
                                    add)
                                nc.scalar.activation(
                                    es[:, j0 * 512 + lo:(j0 + 1) * 512],
                                    tmp[:, lo:512], Exp)
                            else:
                                lo0 = cls0[1]
                                w = len(grp) * 512
                                nc.scalar.activation(
                                    es[:, j0 * 512 + lo0:j0 * 512 + w],
                                    ps_s[:, lo0:w], Exp)
                                for (kt, q2) in grp:
                                    cls = mask_classes[kt][q2]
                                    if cls[0] == 'b':
                                        _, _, mlo, mhi, pid = cls
                                        j = jmap[(kt, q2)]
                                        msl = slice(j * 512 + mlo,
                                                    j * 512 + mhi)
                                        nc.vector.tensor_tensor(
                                            es[:, msl], es[:, msl],
                                            bm_sb[pid], mult)
                    # pass 2: P@V + denominators (ones matmul) + normalize
                    for h in range(HPC):
                        hs = slice(h * HD, (h + 1) * HD)
                        es = es_h[h]
                        for q2 in range(NQ2):
                            lk = live_kt[q2]
                            ps_o = psb.tile([P, 512], F32, tag="ot", bufs=2)
                            for i, (kt, lo) in enumerate(lk):
                                j = jmap[(kt, q2)]
                                nc.tensor.matmul(
                                    ps_o[:, lo:512], lhsT=vb[:, kt, hs],
                                    rhs=es[:, j * 512 + lo:(j + 1) * 512],
                                    start=(i == 0), stop=(i == len(lk) - 1))
                            ps_m = psb.tile([P, 512], F32, tag="sum", bufs=2)
                            for i, (kt, lo) in enumerate(lk):
                                j = jmap[(kt, q2)]
                                nc.tensor.matmul(
                                    ps_m[:, lo:512], lhsT=ones_sb,
                                    rhs=es[:, j * 512 + lo:(j + 1) * 512],
                                    start=(i == 0), stop=(i == len(lk) - 1))
                            rec = tmpb.tile([P, 512], F32, tag="rec", bufs=4)
                            nc.vector.reciprocal_approx_fast(rec, ps_m)
                            ob = stgb.tile([P, 512], MM, tag="ob", bufs=8)
                            nc.vector.tensor_tensor(ob, ps_o, rec, mult)
                            nc.gpsimd.dma_start(
                                bounce[b // BPG][h * HD:(h + 1) * HD,
                                                 (b % BPG) * S + q2 * 512:
                                                 (b % BPG) * S + (q2 + 1) * 512],
                                ob)

                def wo_batch(b):
                    # paired token tiles: 512B DMA lines on ctx gather reads
                    for tt in range(0, S // P, 2):
                        c0 = (b % BPG) * S + tt * P
                        cx = cxp.tile([P, NDK, 2 * P], MM, tag="cx")
                        nc.gpsimd.dma_start(
                            cx, ctxT[b // BPG][:, c0:c0 + 2 * P]
                            .rearrange("(o p) t -> p o t", p=P))
                        ps_y0 = psb.tile([P, CW], F32, tag="ot", bufs=2)
                        ps_y1 = psb.tile([P, CW], F32, tag="sum", bufs=2)
                        for dk in range(NDK):
                            nc.tensor.matmul(
                                ps_y0, lhsT=cx[:, dk, 0:P],
                                rhs=wo_sb[:, dk, :],
                                start=(dk == 0), stop=(dk == NDK - 1))
                            nc.tensor.matmul(
                                ps_y1, lhsT=cx[:, dk, P:2 * P],
                                rhs=wo_sb[:, dk, :],
                                start=(dk == 0), stop=(dk == NDK - 1))
                        for j, ps_y in enumerate((ps_y0, ps_y1)):
                            yo = stgb.tile([P, CW], F32, tag="yo", bufs=2)
                            nc.scalar.copy(yo, ps_y)
                            nc.sync.dma_start(
                                y[b * S + (tt + j) * P:
                                  b * S + (tt + j + 1) * P, :], yo)

                def allgather(i):
                    nc.gpsimd.collective_compute(
                        "AllGather", mybir.AluOpType.bypass,
                        replica_groups=AG_GROUPS,
                        ins=[bounce[i]], outs=[ctxT[i]])

                # software-pipeline: per-batch AllGathers (serialized on the
                # collective stream, ~55us each) hidden under the remaining
                # attention batches and the wo projections. All qkv loads
                # are prefetched a batch ahead on the scalar queue.
                vb0 = qkvp.tile([P, NKT, CW], MM, tag="vb")
                nc.sync.dma_start(
                    vb0, v_d[0:S, :].rearrange("(kt p) w -> p kt w", p=P))
                t1 = qkv_load(1)
                nc.sync.dma_start(wo_sb, woT)
                attn_batch(0, qb0, kb0, vb0)
                allgather(0)
                t2 = qkv_load(2)
                attn_batch(1, *t1)
                allgather(1)
                t3 = qkv_load(3)
                attn_batch(2, *t2)
                allgather(2)
                attn_batch(3, *t3)
                allgather(3)
                wo_batch(0)
                wo_batch(1)
                wo_batch(2)
                wo_batch(3)

    nc.compile()
    return nc


_NC_CACHE = {}


def _get_nc(mask_classes):
    key = tuple(map(tuple, mask_classes))
    if key not in _NC_CACHE:
        _NC_CACHE[key] = build_program(mask_classes)
    return _NC_CACHE[key]


def _classify_mask(maskT_f32):
    """Per score tile [kt*128:(kt+1)*128, q2*512:(q2+1)*512] of mask^T,
    return the class tuple (see build_program) plus 0/1 patterns."""
    classes = []
    patterns = {}
    pat_ids = {}
    for kt in range(NKT):
        row = []
        for q2 in range(NQ2):
            t = maskT_f32[kt * P:(kt + 1) * P, q2 * 512:(q2 + 1) * 512]
            dead_col = np.all(t <= -1e30, axis=0)   # [512]
            if dead_col.all():
                row.append(('d',))
                continue
            live = ~dead_col
            lo = int(np.argmax(live))
            if not live[lo:].all():
                # non-prefix deadness: general fallback
                row.append(('g', 0))
                continue
            sub = t[:, lo:]
            if np.all(sub == 0.0):
                row.append(('z', lo))
                continue
            is_neg = sub <= -1e30
            if not np.all(is_neg | (sub == 0.0)):
                row.append(('g', lo))
                continue
            mixed = is_neg.any(axis=0)              # cols needing 0/1 mask
            m_idx = np.nonzero(mixed)[0]
            mlo, mhi = int(m_idx[0]), int(m_idx[-1]) + 1
            if mixed[mlo:mhi].sum() != mhi - mlo:
                row.append(('g', lo))               # non-contiguous mixed
                continue
            pat = (~is_neg[:, mlo:mhi]).astype(np.float32)
            key = pat.tobytes() + bytes([mhi - mlo])
            if key not in pat_ids:
                pat_ids[key] = len(pat_ids)
                patterns[pat_ids[key]] = np.ascontiguousarray(
                    pat.astype(BF16))
            row.append(('b', lo, lo + mlo, lo + mhi, pat_ids[key]))
        classes.append(row)
    return classes, patterns


def _prep_inputs(x, freqs_cos, freqs_sin, mask, wq, wk, wv, wo):
    """Host-side sharding/layout marshaling. Returns per-core input maps."""
    x = np.asarray(x, np.float32).reshape(T, D)
    xT = np.ascontiguousarray(
        x.T.reshape(NDK, P, T).transpose(1, 0, 2).astype(BF16))

    cos = np.asarray(freqs_cos, np.float32)
    sin = np.asarray(freqs_sin, np.float32)
    qscale = 1.0 / math.sqrt(HD)
    cqh = np.ascontiguousarray(cos.T * qscale).astype(np.float32)
    sqh = np.ascontiguousarray(sin.T * qscale).astype(np.float32)
    ckh = np.ascontiguousarray(cos.T).astype(np.float32)
    skh = np.ascontiguousarray(sin.T).astype(np.float32)

    m = np.asarray(mask, np.float32).reshape(S, S)
    mT = np.ascontiguousarray(m.T)
    classes, patterns = _classify_mask(mT)
    any_g = any(c[0] == 'g' for row in classes for c in row)
    maskTb = np.ascontiguousarray(np.maximum(mT, -60000.0).astype(BF16))

    # deinterleave RoPE pairs within each head's weight rows: row order
    # [0,2,...,126,1,3,...,127] so pairs land in partition blocks.
    perm = np.concatenate([np.arange(0, HD, 2), np.arange(1, HD, 2)])

    wq = np.asarray(wq, np.float32)
    wk = np.asarray(wk, np.float32)
    wv = np.asarray(wv, np.float32)
    wo = np.asarray(wo, np.float32)

    in_maps = []
    for c in range(NCORES):
        r0, r1 = c * CW, (c + 1) * CW
        wq_c = wq[r0:r1].reshape(HPC, HD, D)[:, perm, :].reshape(CW, D)
        wk_c = wk[r0:r1].reshape(HPC, HD, D)[:, perm, :].reshape(CW, D)
        wv_c = wv[r0:r1]
        wo_c = wo[r0:r1]
        im = {
            "xT": xT,
            "wqT": _pretile(wq_c.T),
            "wkT": _pretile(wk_c.T),
            "wvT": _pretile(wv_c.T),
            "woT": _pretile(wo_c.T),
            "cq": cqh, "sq": sqh, "ck": ckh, "sk": skh,
        }
        if any_g:
            im["maskT"] = maskTb
        for pid, pat in patterns.items():
            im[f"bm{pid}"] = pat
        in_maps.append(im)
    return in_maps, classes


def _pretile(wT):
    """[D, CW] -> [P, NDK, CW] with [p, o, m] = wT[o*P + p, m]."""
    return np.ascontiguousarray(
        wT.reshape(NDK, P, CW).transpose(1, 0, 2).astype(BF16))


def kernel(x, start_pos, freqs_cos, freqs_sin, mask, wq, wk, wv, wo,
           cache_k, cache_v, _trace=False):
    assert int(start_pos) == 0, "kernel specialized for start_pos=0"
    in_maps, classes = _prep_inputs(x, freqs_cos, freqs_sin, mask,
                                    wq, wk, wv, wo)
    nc = _get_nc(classes)
    res = run_bass_kernel_spmd(nc, in_maps, list(range(NCORES)), trace=_trace)
    kernel.last_results = res
    yfull = np.concatenate([res.results[c]["y"] for c in range(NCORES)],
                           axis=1)
    return yfull.reshape(B, S, D).astype(np.float32)
